# revision 106
# baseline (speedup 1.0000x reference)
"""NetVLAD-style vq_codebook kernel for 8 Trainium2 NeuronCores.

Reference computation (per full input):
  assn = BN(x @ clusters); softmax over 80 clusters, drop 16 ghosts
  vlad[b,d,k] = sum_n assn[b,n,k] x[b,n,d] - a_sum[b,k]*clusters2[d,k]
  intra-normalize over d, flatten, global L2 normalize -> (B, D*K)

Sharding: data-parallel over batch B (B/8 batches per core). BatchNorm
statistics (sum and sum-of-squares per cluster column) are all-reduced
across the 8 cores (2*80 floats). Everything else is local.

Design notes:
 - x is cast to fp16 on load (gpsimd cast-DMA) in natural token-partition
   layout only. The d-partition copy needed by the assignment matmul is
   produced by PE transposes (128x128 tiles through the PE array, 8 tiles
   packed per PSUM bank via per-address has_written accumulation) and
   drained PSUM->SBUF in 2KB batches rotated across DVE / Act (GPSIMD
   cannot read PSUM). This keeps the DMA engines at ~the x-load cost
   alone, and the PE stream is software-pipelined (transposes of group g,
   drain of g-2, assignments of g-3, Gram two more steps behind) so the
   in-order tensor-engine queue never waits on a drain.
 - BN stats via a Gram matmul: stationary [assn | ones] (81 cols) against
   assn gives out[81, 80] whose row 80 is sum(assn) and whose diagonal is
   sum(assn^2); both are extracted with tiny selector matmuls.
 - BN 1/sqrt(var+eps) is computed on DVE with the quake rsqrt bit-trick
   seed + one Newton iteration (avoids an activation-table load at the
   stats barrier). The intra-norm rsqrt uses Act Sqrt + DVE reciprocal,
   with the sqrt-set table load hoisted behind the last softmax exp.
 - a_sum is accumulated as a PE column [64,1]; a_sum*clusters2 is
   subtracted inside the vlad PSUM accumulation via a diag(-a_sum)
   matmul against a pre-transposed clusters2, so the post stage is a
   plain drain.
 - softmax without max-subtraction (logits are exactly BN-normalized,
   |logit| <~ 6, exp is safe in fp32/fp16).
"""

import sys

for _p in ("/opt/trn_rl_repo", "/root/.axon_site/_ro/trn_rl_repo"):
    if _p not in sys.path:
        sys.path.insert(0, _p)

import numpy as np

import concourse.bacc as bacc
import concourse.mybir as mybir
import concourse.tile as tile
from concourse.bass_utils import run_bass_kernel_spmd

F32 = mybir.dt.float32
F16 = mybir.dt.float16
I32 = mybir.dt.int32
AX = mybir.AxisListType
OP = mybir.AluOpType
ACTF = mybir.ActivationFunctionType

N_CORES = 8
D = 512
KG = 80          # clusters + ghosts
K = 64           # real clusters
N_SEQ = 2048
TPB = N_SEQ // 128   # token tiles per batch = 16
BN_EPS = 1e-5
L2_EPS = 1e-12
MAGIC = 0x5F3759DF


def build(b_loc=4, n_cores=N_CORES, with_collective=True):
    """Build the per-core program. b_loc = batches per core."""
    nt = b_loc * TPB                # token tiles per core
    tok = nt * 128                  # tokens per core
    total_tok = tok * n_cores       # global token count for BN stats
    NG = nt // 4                    # 4-tile groups

    nc = bacc.Bacc("TRN2", target_bir_lowering=False, debug=False,
                   dynamic_dma_scratch_size=65536)

    x = nc.declare_dram_parameter("x", [tok, D], F32, isOutput=False)
    cl = nc.declare_dram_parameter("clusters", [D, KG], F32, isOutput=False)
    c2 = nc.declare_dram_parameter("clusters2", [D, K], F32, isOutput=False)
    gam = nc.declare_dram_parameter("bn_gamma", [1, KG], F32, isOutput=False)
    bet = nc.declare_dram_parameter("bn_beta", [1, KG], F32, isOutput=False)
    # y layout [b, p, c, k] fp16: per-partition-contiguous rows for the
    # output DMA; the host undoes the (p, c) -> (c, p) transpose and
    # upcasts to fp32.
    y = nc.declare_dram_parameter("y", [b_loc, 128, 4 * K], F16,
                                  isOutput=True)

    ones_row_c = nc.inline_tensor(np.ones((1, 128), np.float32), name="c_ones_row")
    ident_c = nc.inline_tensor(np.eye(128, dtype=np.float16), name="c_ident")
    eye80_c = nc.inline_tensor(np.eye(KG, dtype=np.float32), name="c_eye80")
    eye64_c = nc.inline_tensor(np.eye(K, dtype=np.float16), name="c_eye64")
    # selector columns: e80 picks the Gram sums row; ones80 sums the masked diag
    sel_np = np.zeros((KG + 1, 2), np.float32)
    sel_np[KG, 0] = 1.0          # e80
    sel_np[:KG, 1] = 1.0         # ones over rows 0..79
    sel_c = nc.inline_tensor(sel_np, name="c_sel")

    with tile.TileContext(nc) as tc:
        with (
            tc.tile_pool(name="persist", bufs=1) as persist,
            tc.tile_pool(name="work", bufs=4) as work,
            tc.tile_pool(name="dram", bufs=1, space="DRAM") as dram,
        ):
            # ---- persistent SBUF tensors ----
            xh = persist.tile([128, nt, D], F16, name="xh")
            ones16 = persist.tile([128, 1], F16, name="ones16")
            assn = persist.tile([128, nt, KG + 1], F16, name="assn")
            te_all = persist.tile([128, nt, KG], F16, name="te_all")
            sm = persist.tile([128, nt, K], F16, name="sm")
            clh = persist.tile([128, 4, KG], F16, name="clh")
            c2h = persist.tile([128, 4, K], F16, name="c2h")
            c2T = persist.tile([K, 4, 128], F16, name="c2T")
            ident = persist.tile([128, 128], F16, name="ident")
            eye80 = persist.tile([KG, KG], F32, name="eye80")
            eye64 = persist.tile([K, K], F16, name="eye64")
            sel = persist.tile([KG + 1, 2], F32, name="sel")
            ones_row = persist.tile([1, 128], F32, name="ones_row")
            gamma = persist.tile([1, KG], F32, name="gamma")
            beta = persist.tile([1, KG], F32, name="beta")
            ss = persist.tile([1, 2 * KG], F16, name="ss")
            stats_g = persist.tile([1, 2 * KG], F32, name="stats_g")
            bcB = persist.tile([128, 2 * KG], F16, name="bcB")
            nrm2 = persist.tile([1, b_loc * K], F32, name="nrm2")
            vsb = persist.tile([128, b_loc, 4, K], F16, name="vsb")

            # ---- phase 0: constants + x load (natural layout only) ----
            nc.sync.dma_start(ones_row[:], ones_row_c.ap()[:, :])
            nc.sync.dma_start(ident[:], ident_c.ap()[:, :])
            nc.sync.dma_start(eye80[:], eye80_c.ap()[:, :])
            nc.sync.dma_start(eye64[:], eye64_c.ap()[:, :])
            nc.sync.dma_start(sel[:], sel_c.ap()[:, :])
            nc.sync.dma_start(gamma[:], gam[:, :])
            nc.sync.dma_start(beta[:], bet[:, :])
            nc.vector.memset(ones16[:], 1.0)
            # ones column for the Gram stationary
            nc.vector.memset(assn[:, :, KG:], 1.0)

            # x cast-DMA (SWDGE casts fp32->fp16 in the DMA engines; HBM
            # read is the real cost). First groups are small so the PE
            # transpose stream starts early; clusters load between them.
            xr = x.ap().rearrange("(t p) d -> p t d", p=128)
            load_groups = [4] * 14 + [2] * 4
            assert sum(load_groups) == nt
            t0l = 0
            for gi, glen in enumerate(load_groups):
                nc.gpsimd.dma_start(
                    xh[:, t0l:t0l + glen, :], xr[:, t0l:t0l + glen, :])
                t0l += glen
                if gi == 2:
                    # clusters/clusters2 -> fp16 chunks (cast dma)
                    nc.gpsimd.dma_start(
                        clh[:], cl.ap().rearrange("(c p) k -> p c k", p=128))
                    nc.gpsimd.dma_start(
                        c2h[:], c2.ap().rearrange("(c p) k -> p c k", p=128))

            # ---- phase 1: PE transposes + assignment matmuls + Gram stats ----
            NG2 = nt // 2                 # 2-tile transpose groups
            with (
                tc.tile_pool(name="ps_t", bufs=4, space="PSUM") as ps_t,
                tc.tile_pool(name="ps_a", bufs=2, space="PSUM") as ps_a,
                tc.tile_pool(name="ps_g", bufs=1, space="PSUM") as ps_g,
                tc.tile_pool(name="xq", bufs=4) as xqp,
            ):
                gram = ps_g.tile([KG + 1, KG], F32, name="gram", tag="gram",
                                 bufs=1)

                # GPSIMD cannot access PSUM on TRN2 (BIR verifier), so
                # drains rotate over DVE and Act only.
                def drain(eng, dst, src):
                    if eng == 0:
                        nc.vector.tensor_copy(dst, src)
                    else:
                        nc.scalar.activation(dst, src, ACTF.Copy)

                # per 2-tile group: 8 transposes -> 1-bank PSUM tile, one
                # batched drain; 8 assignment matmuls into a 4-tile p14
                # bank; per 4-tile group one p14 drain + 4 Gram matmuls.
                # Pool only drains late groups (it generates SWDGE
                # descriptors for the x loads early on).
                XQ_ROT = [0, 0, 1, 0] * 8
                P14_ROT = [0, 0, 1, 0] * 4
                pts = {}
                p14s = {}
                xqs = {}

                def emit_transposes(g):
                    pt = ps_t.tile([128, 2, 4, 128], F16, name="pt", tag="pt")
                    pts[g] = pt
                    for j in range(2):
                        t = 2 * g + j
                        for c in range(4):
                            nc.tensor.matmul(
                                pt[:, j, c, :],
                                xh[:, t, 128 * c:128 * (c + 1)],
                                ident[:], is_transpose=True,
                                start=(j == 0 and c == 0),
                                stop=(j == 1 and c == 3),
                                skip_group_check=True)

                def emit_drain_t(g):
                    pt = pts.pop(g)
                    xq = xqp.tile([128, 2, 4, 128], F16, name="xqt", tag="xq")
                    xqs[g] = xq
                    drain(XQ_ROT[g], xq[:], pt[:])

                def emit_assign(g):
                    xq = xqs[g]
                    if g % 2 == 0:
                        p14s[g // 2] = ps_a.tile([128, 4, KG], F32,
                                                 name="p14", tag="p14")
                    p14 = p14s[g // 2]
                    for j in range(2):
                        jj = 2 * (g % 2) + j
                        for c in range(4):
                            nc.tensor.matmul(
                                p14[:, jj, :], xq[:, j, c, :], clh[:, c, :],
                                start=(jj == 0 and c == 0),
                                stop=(jj == 3 and c == 3),
                                skip_group_check=True)

                def emit_drain_a(G):
                    p14 = p14s.pop(G)
                    t0 = 4 * G
                    drain(P14_ROT[G % len(P14_ROT)], assn[:, t0:t0 + 4, :KG],
                          p14[:])

                def emit_gram(G):
                    for t in range(4 * G, 4 * G + 4):
                        nc.tensor.matmul(
                            gram[:], assn[:, t, :], assn[:, t, :KG],
                            start=(t == 0), stop=(t == nt - 1),
                            skip_group_check=True)

                # The last two 2-tile groups go through the DMA XBAR: the
                # DMA device is idle once the x loads finish, and this
                # shortens the PE stream right where it trails the loads.
                XBAR_SET = set()   # measured: XBAR offload never pays here

                def emit_xbar_t(g):
                    xq = xqp.tile([128, 2, 4, 128], F16, name="xqt", tag="xq")
                    xqs[g] = xq
                    nc.sync.dma_start(xq[:], xh[:, 2 * g:2 * g + 2, :],
                                      transpose=True)

                # clusters2 transposed (for the diag a_sum matmul in the
                # tail): emitted mid-stream so it fills a load gap
                def emit_c2T():
                    pc2 = ps_t.tile([128, 2, 4, 128], F16, name="pc2",
                                    tag="pt")
                    for c in range(4):
                        nc.tensor.matmul(pc2[:K, 0, c, :], c2h[:, c, :],
                                         ident[:], is_transpose=True,
                                         start=(c == 0), stop=(c == 3),
                                         skip_group_check=True)
                    nc.vector.tensor_copy(c2T[:], pc2[:K, 0, :, :])

                # stagger: transpose(g) | drain(g-2) | assign(g-3) |
                # p14-drain(G at g=2G+5) | gram(G at g=2G+6) — each
                # consumer trails its producer's drain by a full
                # iteration so the in-order PE queue never stalls.
                for g in range(NG2 + 7):
                    if g == NG2:
                        emit_c2T()
                    if g < NG2:
                        if g in XBAR_SET:
                            emit_xbar_t(g)
                        else:
                            emit_transposes(g)
                    if 2 <= g < NG2 + 2 and g - 2 not in XBAR_SET:
                        emit_drain_t(g - 2)
                    if 3 <= g < NG2 + 3:
                        emit_assign(g - 3)
                    if 5 <= g and (g - 5) % 2 == 0 and (g - 5) // 2 < NG2 // 2:
                        emit_drain_a((g - 5) // 2)
                    if 6 <= g and (g - 6) % 2 == 0 and (g - 6) // 2 < NG2 // 2:
                        G = (g - 6) // 2
                        emit_gram(G)
                        xqs.pop(2 * G, None)
                        xqs.pop(2 * G + 1, None)

                # ---- phase 2: stats extraction ----
                gram_sb = work.tile([KG + 1, KG], F32, name="gram_sb",
                                    tag="gsb", bufs=1)
                masked = work.tile([KG, KG], F32, name="masked", tag="msk",
                                   bufs=1)
                nc.vector.tensor_copy(gram_sb[:], gram[:])
                nc.vector.tensor_tensor(masked[:], gram_sb[:KG, :], eye80[:],
                                        op=OP.mult)
                psst = ps_a.tile([1, 2 * KG], F32, name="psst", tag="psst",
                                 bufs=1)
                # sums = Gram row 80 (selector e80); sumsq = sum of masked diag
                nc.tensor.matmul(psst[:, :KG], sel[:, 0:1], gram_sb[:],
                                 start=True, stop=False, skip_group_check=True)
                nc.tensor.matmul(psst[:, KG:], sel[:KG, 1:2], masked[:],
                                 start=False, stop=True, skip_group_check=True)

                if with_collective:
                    stats_in = dram.tile([1, 2 * KG], F32, name="stats_in")
                    stats_out = dram.tile([1, 2 * KG], F32, name="stats_out")
                    stats_sb = work.tile([1, 2 * KG], F32, name="stats_sb",
                                         tag="ssb", bufs=1)
                    nc.vector.tensor_copy(stats_sb[:], psst[:])
                    nc.sync.dma_start(stats_in[:], stats_sb[:])
                    nc.gpsimd.collective_compute(
                        "AllReduce", OP.add,
                        replica_groups=[list(range(n_cores))],
                        ins=[stats_in.opt()], outs=[stats_out.opt()])
                    nc.sync.dma_start(stats_g[:], stats_out[:])
                else:
                    # single-core allreduce is the identity
                    nc.vector.tensor_copy(stats_g[:], psst[:])

            # ---- phase 2b: BN scale/shift (all on DVE; rsqrt via the
            # quake bit-trick seed + 2 Newton iterations) ----
            t_mean = work.tile([1, KG], F32, name="t_mean", tag="sv", bufs=8)
            t_var = work.tile([1, KG], F32, name="t_var", tag="sv", bufs=8)
            t_y = work.tile([1, KG], F32, name="t_y", tag="sv", bufs=8)
            t_t = work.tile([1, KG], F32, name="t_t", tag="sv", bufs=8)
            t_ms = work.tile([1, KG], F32, name="t_ms", tag="sv", bufs=8)
            inv_n = 1.0 / float(total_tok)
            nc.vector.tensor_scalar_mul(t_mean[:], stats_g[:, :KG], inv_n)
            # E[x^2] + eps in one op; var+eps = that - mean^2
            nc.vector.tensor_scalar(t_var[:], stats_g[:, KG:], inv_n, BN_EPS,
                                    op0=OP.mult, op1=OP.add)
            nc.vector.tensor_tensor(t_ms[:], t_mean[:], t_mean[:], op=OP.mult)
            nc.vector.tensor_tensor(t_var[:], t_var[:], t_ms[:], op=OP.subtract)
            # rsqrt(var+eps): y0 = bits magic, then 2x Newton
            nc.vector.tensor_scalar(t_y[:].bitcast(I32), t_var[:].bitcast(I32),
                                    1, -1, op0=OP.logical_shift_right,
                                    op1=OP.bitwise_xor)
            nc.vector.tensor_scalar(t_y[:].bitcast(I32), t_y[:].bitcast(I32),
                                    MAGIC + 1, None, op0=OP.add)
            for _ in range(1):  # 1 Newton iteration: ~2e-3 relative
                nc.vector.tensor_tensor(t_t[:], t_y[:], t_y[:], op=OP.mult)
                nc.vector.tensor_tensor(t_t[:], t_var[:], t_t[:], op=OP.mult)
                nc.vector.tensor_scalar(t_t[:], t_t[:], -0.5, 1.5,
                                        op0=OP.mult, op1=OP.add)
                nc.vector.tensor_tensor(t_y[:], t_y[:], t_t[:], op=OP.mult)
            nc.vector.tensor_tensor(ss[:, :KG], t_y[:], gamma[:], op=OP.mult)
            # broadcast the scale while the shift is still being computed
            nc.gpsimd.partition_broadcast(bcB[:, :KG], ss[:, :KG])
            nc.vector.tensor_tensor(t_ms[:], t_mean[:], ss[:, :KG], op=OP.mult)
            nc.vector.tensor_tensor(ss[:, KG:], beta[:], t_ms[:], op=OP.subtract)
            nc.gpsimd.partition_broadcast(bcB[:, KG:], ss[:, KG:])

            # ---- phase 3: per-batch softmax + vlad + normalization ----
            with (
                tc.tile_pool(name="ps2", bufs=2, space="PSUM") as ps2,
                tc.tile_pool(name="vpost", bufs=3) as vpost,
            ):
                scale_b = bcB[:, :KG].rearrange("p (a k) -> p a k", a=1)
                shift_b = bcB[:, KG:].rearrange("p (a k) -> p a k", a=1)

                # softmax + vlad run in half-batch units of 8 token tiles,
                # stage-interleaved so the in-order DVE queue never head-
                # blocks on the Act exp of the previous unit.
                HU = TPB // 2
                state = {}

                def sm_aff(u):
                    t0 = u * HU
                    te = te_all[:, t0:t0 + HU, :]
                    nc.vector.tensor_tensor(
                        te, assn[:, t0:t0 + HU, :KG],
                        scale_b.to_broadcast([128, HU, KG]), op=OP.mult)
                    nc.vector.tensor_tensor(
                        te, te, shift_b.to_broadcast([128, HU, KG]),
                        op=OP.add)
                    nc.scalar.activation(te, te, ACTF.Exp)

                def sm_den(u):
                    t0 = u * HU
                    te = te_all[:, t0:t0 + HU, :]
                    denom = work.tile([128, HU], F16, name="denom", tag="dn")
                    with nc.allow_low_precision("fp16 softmax denom"):
                        nc.vector.tensor_reduce(denom[:], te, axis=AX.X,
                                                op=OP.add)
                    recip = work.tile([128, HU], F16, name="recip", tag="rc")
                    with nc.allow_low_precision("fp16 softmax recip"):
                        nc.vector.reciprocal(recip[:], denom[:])
                    # sm-mult on gpsimd to unload DVE; the last unit's
                    # goes on DVE (idle by then) to shorten the end chain
                    eng = nc.vector if u == 2 * b_loc - 1 else nc.gpsimd
                    eng.tensor_tensor(
                        sm[:, t0:t0 + HU, :], te[:, :, :K],
                        recip[:].rearrange("p (t a) -> p t a", a=1)
                        .to_broadcast([128, HU, K]), op=OP.mult)

                def mm_unit(u):
                    b, h = divmod(u, 2)
                    t0 = u * HU
                    if h == 0:
                        pv2 = ps2.tile([128, 4 * K], F32, name="pv2",
                                       tag="pv")
                        pac = ps2.tile([K, 1], F32, name="pac", tag="pac")
                        state[b] = [pv2, pac, None]
                    pv2, pac, _ = state[b]
                    pv3 = pv2[:].rearrange("p (c k) -> p c k", c=4)
                    for c in range(4):
                        for i in range(HU):
                            t = t0 + i
                            nc.tensor.matmul(
                                pv3[:, c, :],
                                xh[:, t, c * 128:(c + 1) * 128],
                                sm[:, t, :],
                                start=(h == 0 and i == 0 and c == 0),
                                stop=False, skip_group_check=True)
                    # a_sum as a PE column: accumulating 1-row matmuls
                    for i in range(HU):
                        nc.tensor.matmul(pac[:], sm[:, t0 + i, :], ones16[:],
                                         start=(h == 0 and i == 0),
                                         stop=(h == 1 and i == HU - 1),
                                         skip_group_check=True)
                    if h == 1:
                        # -a_sum (fp16, SBUF; Act negating copy from PSUM)
                        # and diag(-a_sum) on gpsimd
                        acol = work.tile([K, 1], F16, name="acol", tag="acol")
                        nc.vector.tensor_scalar_mul(acol[:], pac[:], -1.0)
                        dga = vpost.tile([K, K], F16, name="dga", tag="dga")
                        nc.gpsimd.tensor_tensor(
                            dga[:], eye64[:], acol[:].to_broadcast([K, K]),
                            op=OP.mult)
                        state[b][2] = dga

                def close_batch(b):
                    pv2, _, dga = state[b]
                    pv3 = pv2[:].rearrange("p (c k) -> p c k", c=4)
                    for c in range(4):
                        nc.tensor.matmul(pv3[:, c, :], c2T[:, c, :], dga[:],
                                         start=False, stop=(c == 3),
                                         skip_group_check=True)

                def post_stage(b, last=False):
                    pv2, _, _ = state.pop(b)
                    pv3 = pv2[:].rearrange("p (c k) -> p c k", c=4)
                    v = vsb[:, b, :, :]
                    with nc.allow_low_precision("fp16 vlad drain"):
                        nc.scalar.activation(v, pv3[:], ACTF.Copy)
                    sq = vpost.tile([128, 4, K], F16, name="sq", tag="sq")
                    with nc.allow_low_precision("fp16 norm squares"):
                        if last:
                            # off the serial tail: square straight from
                            # PSUM on DVE, parallel with the Act drain
                            nc.vector.tensor_copy(sq[:], pv3[:])
                            nc.vector.tensor_tensor(sq[:], sq[:], sq[:],
                                                    op=OP.mult)
                        else:
                            nc.scalar.square(sq[:], v)
                    # intra-norm^2 per k: 4 accumulating ones-matmuls
                    pnrm = ps2.tile([1, K], F32, name="pnrm", tag="pnrm")
                    for c in range(4):
                        nc.tensor.matmul(pnrm[:], ones16[:], sq[:, c, :],
                                         start=(c == 0), stop=(c == 3),
                                         skip_group_check=True)
                    # 64*nrm2 (+tiny guard) for rn = 1/(8*sqrt(nrm2))
                    nc.vector.tensor_scalar(nrm2[:, b * K:(b + 1) * K],
                                            pnrm[:], 64.0, 64.0 * L2_EPS**2,
                                            op0=OP.mult, op1=OP.add)

                sn = persist.tile([1, b_loc * K], F32, name="sn")
                rn = persist.tile([1, b_loc * K], F16, name="rn")

                def finale_part(b0f, b1f):
                    # rn = 1/sqrt(64*nrm2); global L2 norm of the
                    # intra-normalized vlad is exactly sqrt(K)=8 (each of
                    # the K columns has unit norm; the eps guard can only
                    # fire on an exactly-zero column, which cannot occur
                    # for nondegenerate inputs). Batches 0..b_loc-2 are
                    # finished as soon as their norms land (the sqrt-set
                    # table is loaded right after the last exp); only the
                    # last batch's short chain sits at the end.
                    sl = slice(b0f * K, b1f * K)
                    nc.scalar.sqrt(sn[:, sl], nrm2[:, sl])
                    with nc.allow_low_precision("fp16 norm recip"):
                        nc.vector.reciprocal(rn[:, sl], sn[:, sl])
                    for b in range(b0f, b1f):
                        rn_sb = work.tile([128, K], F16, name="rn_sb",
                                          tag="rnsb")
                        nc.gpsimd.partition_broadcast(
                            rn_sb[:], rn[:, b * K:(b + 1) * K])
                        vb = vsb[:, b, :, :]
                        rb = (rn_sb[:].rearrange("p (a k) -> p a k", a=1)
                              .to_broadcast([128, 4, K]))
                        with nc.allow_low_precision("fp16 final scale"):
                            nc.vector.tensor_tensor(vb, vb, rb, op=OP.mult)

                def finale():
                    finale_part(b_loc - 1, b_loc)
                    # one output DMA for all batches: single HWDGE
                    # overhead, full-bandwidth descriptors
                    nc.sync.dma_start(
                        y.ap().rearrange("b p f -> p b f"), vsb[:])

                dum = work.tile([1, 1], F32, name="dum", tag="dum", bufs=1)
                NU = 2 * b_loc
                sm_aff(0)
                for u in range(NU):
                    if u + 1 < NU:
                        sm_aff(u + 1)
                        if u + 1 == NU - 1:
                            # dummy sqrt right behind the last exp: the
                            # sqrt-set activation-table load runs as soon
                            # as that exp retires, off the final serial
                            # chain (copy/square live in that set too)
                            nc.scalar.sqrt(dum[:], ones_row[:, :1])
                    sm_den(u)
                    mm_unit(u)
                    b, h = divmod(u, 2)
                    if h == 1 and b >= 1:
                        close_batch(b - 1)
                        post_stage(b - 1)
                        if b == b_loc - 1:
                            finale_part(0, b_loc - 1)
                close_batch(b_loc - 1)
                post_stage(b_loc - 1, last=True)
                finale()
    nc.compile()
    return nc


_CACHE = {}


def _get(b_loc, n_cores, with_collective):
    key = (b_loc, n_cores, with_collective)
    if key not in _CACHE:
        _CACHE[key] = build(b_loc, n_cores, with_collective)
    return _CACHE[key]


def make_in_maps(x, clusters, clusters2, bn_gamma, bn_beta, n_cores=N_CORES):
    B = x.shape[0]
    b_loc = B // n_cores
    shared = {
        "clusters": np.ascontiguousarray(clusters, np.float32),
        "clusters2": np.ascontiguousarray(
            np.asarray(clusters2).reshape(D, K), np.float32),
        "bn_gamma": np.ascontiguousarray(
            np.asarray(bn_gamma).reshape(1, KG), np.float32),
        "bn_beta": np.ascontiguousarray(
            np.asarray(bn_beta).reshape(1, KG), np.float32),
    }
    in_maps = []
    for i in range(n_cores):
        m = dict(shared)
        m["x"] = np.ascontiguousarray(
            np.asarray(x[i * b_loc:(i + 1) * b_loc]).reshape(
                b_loc * N_SEQ, D), np.float32)
        in_maps.append(m)
    return in_maps


def kernel(x, clusters, clusters2, bn_gamma, bn_beta):
    B, N, Dd = x.shape
    assert (N, Dd) == (N_SEQ, D) and B % N_CORES == 0
    b_loc = B // N_CORES
    nc = _get(b_loc, N_CORES, True)
    in_maps = make_in_maps(x, clusters, clusters2, bn_gamma, bn_beta)
    res = run_bass_kernel_spmd(nc, in_maps, core_ids=list(range(N_CORES)))
    out = np.concatenate([res.results[i]["y"] for i in range(N_CORES)], axis=0)
    # y is [b, p, c, k] fp16; vlad flat index is (128c+p)*K + k
    out = out.astype(np.float32)
    out = out.reshape(B, 128, 4, K).transpose(0, 2, 1, 3).reshape(B, D * K)
    return np.ascontiguousarray(out)


# revision 111
# speedup vs baseline: 1.0013x; 1.0013x over previous
"""NetVLAD-style vq_codebook kernel for 8 Trainium2 NeuronCores.

Reference computation (per full input):
  assn = BN(x @ clusters); softmax over 80 clusters, drop 16 ghosts
  vlad[b,d,k] = sum_n assn[b,n,k] x[b,n,d] - a_sum[b,k]*clusters2[d,k]
  intra-normalize over d, flatten, global L2 normalize -> (B, D*K)

Sharding: data-parallel over batch B (B/8 batches per core). BatchNorm
statistics (sum and sum-of-squares per cluster column) are all-reduced
across the 8 cores (2*80 floats). Everything else is local.

Design notes:
 - x is cast to fp16 on load (gpsimd cast-DMA) in natural token-partition
   layout only. The d-partition copy needed by the assignment matmul is
   produced by PE transposes (128x128 tiles through the PE array, 8 tiles
   packed per PSUM bank via per-address has_written accumulation) and
   drained PSUM->SBUF in 2KB batches rotated across DVE / Act (GPSIMD
   cannot read PSUM). This keeps the DMA engines at ~the x-load cost
   alone, and the PE stream is software-pipelined (transposes of group g,
   drain of g-2, assignments of g-3, Gram two more steps behind) so the
   in-order tensor-engine queue never waits on a drain.
 - BN stats via a Gram matmul: stationary [assn | ones] (81 cols) against
   assn gives out[81, 80] whose row 80 is sum(assn) and whose diagonal is
   sum(assn^2); both are extracted with tiny selector matmuls.
 - BN 1/sqrt(var+eps) is computed on DVE with the quake rsqrt bit-trick
   seed + one Newton iteration (avoids an activation-table load at the
   stats barrier). The intra-norm rsqrt uses Act Sqrt + DVE reciprocal,
   with the sqrt-set table load hoisted behind the last softmax exp.
 - a_sum is accumulated as a PE column [64,1]; a_sum*clusters2 is
   subtracted inside the vlad PSUM accumulation via a diag(-a_sum)
   matmul against a pre-transposed clusters2, so the post stage is a
   plain drain.
 - softmax without max-subtraction (logits are exactly BN-normalized,
   |logit| <~ 6, exp is safe in fp32/fp16).
"""

import sys

for _p in ("/opt/trn_rl_repo", "/root/.axon_site/_ro/trn_rl_repo"):
    if _p not in sys.path:
        sys.path.insert(0, _p)

import numpy as np

import concourse.bacc as bacc
import concourse.mybir as mybir
import concourse.tile as tile
from concourse.bass_utils import run_bass_kernel_spmd

F32 = mybir.dt.float32
F16 = mybir.dt.float16
I32 = mybir.dt.int32
AX = mybir.AxisListType
OP = mybir.AluOpType
ACTF = mybir.ActivationFunctionType

N_CORES = 8
D = 512
KG = 80          # clusters + ghosts
K = 64           # real clusters
N_SEQ = 2048
TPB = N_SEQ // 128   # token tiles per batch = 16
BN_EPS = 1e-5
L2_EPS = 1e-12
MAGIC = 0x5F3759DF


def build(b_loc=4, n_cores=N_CORES, with_collective=True):
    """Build the per-core program. b_loc = batches per core."""
    nt = b_loc * TPB                # token tiles per core
    tok = nt * 128                  # tokens per core
    total_tok = tok * n_cores       # global token count for BN stats
    NG = nt // 4                    # 4-tile groups

    nc = bacc.Bacc("TRN2", target_bir_lowering=False, debug=False,
                   dynamic_dma_scratch_size=65536)

    x = nc.declare_dram_parameter("x", [tok, D], F32, isOutput=False)
    cl = nc.declare_dram_parameter("clusters", [D, KG], F32, isOutput=False)
    c2 = nc.declare_dram_parameter("clusters2", [D, K], F32, isOutput=False)
    gam = nc.declare_dram_parameter("bn_gamma", [1, KG], F32, isOutput=False)
    bet = nc.declare_dram_parameter("bn_beta", [1, KG], F32, isOutput=False)
    # y layout [b, p, c, k] fp16: per-partition-contiguous rows for the
    # output DMA; the host undoes the (p, c) -> (c, p) transpose and
    # upcasts to fp32.
    y = nc.declare_dram_parameter("y", [b_loc, 128, 4 * K], F16,
                                  isOutput=True)

    ones_row_c = nc.inline_tensor(np.ones((1, 128), np.float32), name="c_ones_row")
    ident_c = nc.inline_tensor(np.eye(128, dtype=np.float16), name="c_ident")
    eye80_c = nc.inline_tensor(np.eye(KG, dtype=np.float32), name="c_eye80")
    eye64_c = nc.inline_tensor(np.eye(K, dtype=np.float16), name="c_eye64")
    # selector columns: e80 picks the Gram sums row; ones80 sums the masked diag
    sel_np = np.zeros((KG + 1, 2), np.float32)
    sel_np[KG, 0] = 1.0          # e80
    sel_np[:KG, 1] = 1.0         # ones over rows 0..79
    sel_c = nc.inline_tensor(sel_np, name="c_sel")

    with tile.TileContext(nc) as tc:
        with (
            tc.tile_pool(name="persist", bufs=1) as persist,
            tc.tile_pool(name="work", bufs=4) as work,
            tc.tile_pool(name="dram", bufs=1, space="DRAM") as dram,
        ):
            # ---- persistent SBUF tensors ----
            xh = persist.tile([128, nt, D], F16, name="xh")
            ones16 = persist.tile([128, 1], F16, name="ones16")
            assn = persist.tile([128, nt, KG + 1], F16, name="assn")
            te_all = persist.tile([128, nt, KG], F16, name="te_all")
            sm = persist.tile([128, nt, K], F16, name="sm")
            clh = persist.tile([128, 4, KG], F16, name="clh")
            c2h = persist.tile([128, 4, K], F16, name="c2h")
            c2T = persist.tile([K, 4, 128], F16, name="c2T")
            ident = persist.tile([128, 128], F16, name="ident")
            eye80 = persist.tile([KG, KG], F32, name="eye80")
            eye64 = persist.tile([K, K], F16, name="eye64")
            sel = persist.tile([KG + 1, 2], F32, name="sel")
            ones_row = persist.tile([1, 128], F32, name="ones_row")
            gamma = persist.tile([1, KG], F32, name="gamma")
            beta = persist.tile([1, KG], F32, name="beta")
            ss = persist.tile([1, 2 * KG], F16, name="ss")
            stats_g = persist.tile([1, 2 * KG], F32, name="stats_g")
            bcB = persist.tile([128, 2 * KG], F16, name="bcB")
            nrm2 = persist.tile([1, b_loc * K], F32, name="nrm2")
            vsb = persist.tile([128, b_loc, 4, K], F16, name="vsb")

            # ---- phase 0: constants + x load (natural layout only) ----
            nc.sync.dma_start(ones_row[:], ones_row_c.ap()[:, :])
            nc.sync.dma_start(ident[:], ident_c.ap()[:, :])
            nc.sync.dma_start(eye80[:], eye80_c.ap()[:, :])
            nc.sync.dma_start(eye64[:], eye64_c.ap()[:, :])
            nc.sync.dma_start(sel[:], sel_c.ap()[:, :])
            nc.sync.dma_start(gamma[:], gam[:, :])
            nc.sync.dma_start(beta[:], bet[:, :])
            nc.vector.memset(ones16[:], 1.0)
            # ones column for the Gram stationary
            nc.vector.memset(assn[:, :, KG:], 1.0)

            # x cast-DMA (SWDGE casts fp32->fp16 in the DMA engines; HBM
            # read is the real cost). First groups are small so the PE
            # transpose stream starts early; clusters load between them.
            xr = x.ap().rearrange("(t p) d -> p t d", p=128)
            load_groups = [4] * 14 + [2] * 4
            assert sum(load_groups) == nt
            t0l = 0
            for gi, glen in enumerate(load_groups):
                nc.gpsimd.dma_start(
                    xh[:, t0l:t0l + glen, :], xr[:, t0l:t0l + glen, :])
                t0l += glen
                if gi == 2:
                    # clusters/clusters2 -> fp16 chunks (cast dma)
                    nc.gpsimd.dma_start(
                        clh[:], cl.ap().rearrange("(c p) k -> p c k", p=128))
                    nc.gpsimd.dma_start(
                        c2h[:], c2.ap().rearrange("(c p) k -> p c k", p=128))

            t_mean = work.tile([1, KG], F32, name="t_mean", tag="sv", bufs=8)
            t_var = work.tile([1, KG], F32, name="t_var", tag="sv", bufs=8)
            t_y = work.tile([1, KG], F32, name="t_y", tag="sv", bufs=8)
            t_t = work.tile([1, KG], F32, name="t_t", tag="sv", bufs=8)
            t_ms = work.tile([1, KG], F32, name="t_ms", tag="sv", bufs=8)
            inv_n = 1.0 / float(total_tok)

            # ---- phase 1: PE transposes + assignment matmuls + Gram stats ----
            NG2 = nt // 2                 # 2-tile transpose groups
            with (
                tc.tile_pool(name="ps_t", bufs=4, space="PSUM") as ps_t,
                tc.tile_pool(name="ps_a", bufs=2, space="PSUM") as ps_a,
                tc.tile_pool(name="ps_g", bufs=1, space="PSUM") as ps_g,
                tc.tile_pool(name="xq", bufs=4) as xqp,
            ):
                gram = ps_g.tile([KG + 1, KG], F32, name="gram", tag="gram",
                                 bufs=1)

                # GPSIMD cannot access PSUM on TRN2 (BIR verifier), so
                # drains rotate over DVE and Act only.
                def drain(eng, dst, src):
                    if eng == 0:
                        nc.vector.tensor_copy(dst, src)
                    else:
                        nc.scalar.activation(dst, src, ACTF.Copy)

                # per 2-tile group: 8 transposes -> 1-bank PSUM tile, one
                # batched drain; 8 assignment matmuls into a 4-tile p14
                # bank; per 4-tile group one p14 drain + 4 Gram matmuls.
                # Pool only drains late groups (it generates SWDGE
                # descriptors for the x loads early on).
                XQ_ROT = [0, 0, 1, 0] * 8
                P14_ROT = [0, 0, 1, 0] * 4
                pts = {}
                p14s = {}
                xqs = {}

                def emit_transposes(g):
                    pt = ps_t.tile([128, 2, 4, 128], F16, name="pt", tag="pt")
                    pts[g] = pt
                    for j in range(2):
                        t = 2 * g + j
                        for c in range(4):
                            nc.tensor.matmul(
                                pt[:, j, c, :],
                                xh[:, t, 128 * c:128 * (c + 1)],
                                ident[:], is_transpose=True,
                                start=(j == 0 and c == 0),
                                stop=(j == 1 and c == 3),
                                skip_group_check=True)

                def emit_drain_t(g):
                    pt = pts.pop(g)
                    xq = xqp.tile([128, 2, 4, 128], F16, name="xqt", tag="xq")
                    xqs[g] = xq
                    drain(XQ_ROT[g], xq[:], pt[:])

                def emit_assign(g):
                    xq = xqs[g]
                    if g % 2 == 0:
                        p14s[g // 2] = ps_a.tile([128, 4, KG], F32,
                                                 name="p14", tag="p14")
                    p14 = p14s[g // 2]
                    for j in range(2):
                        jj = 2 * (g % 2) + j
                        for c in range(4):
                            nc.tensor.matmul(
                                p14[:, jj, :], xq[:, j, c, :], clh[:, c, :],
                                start=(jj == 0 and c == 0),
                                stop=(jj == 3 and c == 3),
                                skip_group_check=True)

                def emit_drain_a(G):
                    p14 = p14s.pop(G)
                    t0 = 4 * G
                    drain(P14_ROT[G % len(P14_ROT)], assn[:, t0:t0 + 4, :KG],
                          p14[:])

                def emit_gram(G):
                    for t in range(4 * G, 4 * G + 4):
                        nc.tensor.matmul(
                            gram[:], assn[:, t, :], assn[:, t, :KG],
                            start=(t == 0), stop=(t == nt - 1),
                            skip_group_check=True)

                # The last two 2-tile groups go through the DMA XBAR: the
                # DMA device is idle once the x loads finish, and this
                # shortens the PE stream right where it trails the loads.
                XBAR_SET = set()   # measured: XBAR offload never pays here

                def emit_xbar_t(g):
                    xq = xqp.tile([128, 2, 4, 128], F16, name="xqt", tag="xq")
                    xqs[g] = xq
                    nc.sync.dma_start(xq[:], xh[:, 2 * g:2 * g + 2, :],
                                      transpose=True)

                # clusters2 transposed (for the diag a_sum matmul in the
                # tail): emitted mid-stream so it fills a load gap
                def emit_c2T():
                    pc2 = ps_t.tile([128, 2, 4, 128], F16, name="pc2",
                                    tag="pt")
                    for c in range(4):
                        nc.tensor.matmul(pc2[:K, 0, c, :], c2h[:, c, :],
                                         ident[:], is_transpose=True,
                                         start=(c == 0), stop=(c == 3),
                                         skip_group_check=True)
                    nc.vector.tensor_copy(c2T[:], pc2[:K, 0, :, :])

                # stagger: transpose(g) | drain(g-2) | assign(g-3) |
                # p14-drain(G at g=2G+5) | gram(G at g=2G+6) — each
                # consumer trails its producer's drain by a full
                # iteration so the in-order PE queue never stalls.
                for g in range(NG2 + 7):
                    if g == NG2:
                        emit_c2T()
                    if g < NG2:
                        if g in XBAR_SET:
                            emit_xbar_t(g)
                        else:
                            emit_transposes(g)
                    if 2 <= g < NG2 + 2 and g - 2 not in XBAR_SET:
                        emit_drain_t(g - 2)
                    if 3 <= g < NG2 + 3:
                        emit_assign(g - 3)
                    if 5 <= g and (g - 5) % 2 == 0 and (g - 5) // 2 < NG2 // 2:
                        emit_drain_a((g - 5) // 2)
                    if 6 <= g and (g - 6) % 2 == 0 and (g - 6) // 2 < NG2 // 2:
                        G = (g - 6) // 2
                        emit_gram(G)
                        xqs.pop(2 * G, None)
                        xqs.pop(2 * G + 1, None)

                # ---- phase 2: stats extraction ----
                gram_sb = work.tile([KG + 1, KG], F32, name="gram_sb",
                                    tag="gsb", bufs=1)
                masked = work.tile([KG, KG], F32, name="masked", tag="msk",
                                   bufs=1)
                nc.vector.tensor_copy(gram_sb[:], gram[:])
                nc.vector.tensor_tensor(masked[:], gram_sb[:KG, :], eye80[:],
                                        op=OP.mult)
                psst = ps_a.tile([1, 2 * KG], F32, name="psst", tag="psst",
                                 bufs=1)
                # sums = Gram row 80 (selector e80); sumsq = sum of masked diag
                nc.tensor.matmul(psst[:, :KG], sel[:, 0:1], gram_sb[:],
                                 start=True, stop=False, skip_group_check=True)
                nc.tensor.matmul(psst[:, KG:], sel[:KG, 1:2], masked[:],
                                 start=False, stop=True, skip_group_check=True)

                if with_collective:
                    stats_in = dram.tile([1, 2 * KG], F32, name="stats_in")
                    stats_out = dram.tile([1, 2 * KG], F32, name="stats_out")
                    stats_sb = work.tile([1, 2 * KG], F32, name="stats_sb",
                                         tag="ssb", bufs=1)
                    nc.vector.tensor_copy(stats_sb[:], psst[:])
                    nc.sync.dma_start(stats_in[:], stats_sb[:])
                    nc.gpsimd.collective_compute(
                        "AllReduce", OP.add,
                        replica_groups=[list(range(n_cores))],
                        ins=[stats_in.opt()], outs=[stats_out.opt()])
                    nc.sync.dma_start(stats_g[:], stats_out[:])
                else:
                    # single-core allreduce is the identity: fold the
                    # 1/N scaling into the PSUM read, skipping the copy
                    nc.vector.tensor_scalar_mul(t_mean[:], psst[:, :KG],
                                                inv_n)
                    nc.vector.tensor_scalar(t_var[:], psst[:, KG:], inv_n,
                                            BN_EPS, op0=OP.mult, op1=OP.add)

            # ---- phase 2b: BN scale/shift (all on DVE; rsqrt via the
            # quake bit-trick seed + 2 Newton iterations) ----
            if with_collective:
                nc.vector.tensor_scalar_mul(t_mean[:], stats_g[:, :KG],
                                            inv_n)
                # E[x^2] + eps in one op; var+eps = that - mean^2
                nc.vector.tensor_scalar(t_var[:], stats_g[:, KG:], inv_n,
                                        BN_EPS, op0=OP.mult, op1=OP.add)
            nc.vector.tensor_tensor(t_ms[:], t_mean[:], t_mean[:], op=OP.mult)
            nc.vector.tensor_tensor(t_var[:], t_var[:], t_ms[:], op=OP.subtract)
            # rsqrt(var+eps): y0 = bits magic, then 2x Newton
            nc.vector.tensor_scalar(t_y[:].bitcast(I32), t_var[:].bitcast(I32),
                                    1, -1, op0=OP.logical_shift_right,
                                    op1=OP.bitwise_xor)
            nc.vector.tensor_scalar(t_y[:].bitcast(I32), t_y[:].bitcast(I32),
                                    MAGIC + 1, None, op0=OP.add)
            for _ in range(1):  # 1 Newton iteration: ~2e-3 relative
                nc.vector.tensor_tensor(t_t[:], t_y[:], t_y[:], op=OP.mult)
                nc.vector.tensor_tensor(t_t[:], t_var[:], t_t[:], op=OP.mult)
                nc.vector.tensor_scalar(t_t[:], t_t[:], -0.5, 1.5,
                                        op0=OP.mult, op1=OP.add)
                nc.vector.tensor_tensor(t_y[:], t_y[:], t_t[:], op=OP.mult)
            nc.vector.tensor_tensor(ss[:, :KG], t_y[:], gamma[:], op=OP.mult)
            # broadcast the scale while the shift is still being computed
            nc.gpsimd.partition_broadcast(bcB[:, :KG], ss[:, :KG])
            nc.vector.tensor_tensor(t_ms[:], t_mean[:], ss[:, :KG], op=OP.mult)
            nc.vector.tensor_tensor(ss[:, KG:], beta[:], t_ms[:], op=OP.subtract)
            nc.gpsimd.partition_broadcast(bcB[:, KG:], ss[:, KG:])

            # ---- phase 3: per-batch softmax + vlad + normalization ----
            with (
                tc.tile_pool(name="ps2", bufs=2, space="PSUM") as ps2,
                tc.tile_pool(name="vpost", bufs=3) as vpost,
            ):
                scale_b = bcB[:, :KG].rearrange("p (a k) -> p a k", a=1)
                shift_b = bcB[:, KG:].rearrange("p (a k) -> p a k", a=1)

                # softmax + vlad run in half-batch units of 8 token tiles,
                # stage-interleaved so the in-order DVE queue never head-
                # blocks on the Act exp of the previous unit.
                HU = TPB // 2
                state = {}

                def sm_aff(u):
                    t0 = u * HU
                    te = te_all[:, t0:t0 + HU, :]
                    nc.vector.tensor_tensor(
                        te, assn[:, t0:t0 + HU, :KG],
                        scale_b.to_broadcast([128, HU, KG]), op=OP.mult)
                    nc.vector.tensor_tensor(
                        te, te, shift_b.to_broadcast([128, HU, KG]),
                        op=OP.add)
                    nc.scalar.activation(te, te, ACTF.Exp)

                def sm_den(u):
                    t0 = u * HU
                    te = te_all[:, t0:t0 + HU, :]
                    denom = work.tile([128, HU], F16, name="denom", tag="dn")
                    with nc.allow_low_precision("fp16 softmax denom"):
                        nc.vector.tensor_reduce(denom[:], te, axis=AX.X,
                                                op=OP.add)
                    recip = work.tile([128, HU], F16, name="recip", tag="rc")
                    with nc.allow_low_precision("fp16 softmax recip"):
                        nc.vector.reciprocal(recip[:], denom[:])
                    # sm-mult on gpsimd to unload DVE; the last unit's
                    # goes on DVE (idle by then) to shorten the end chain
                    eng = nc.vector if u == 2 * b_loc - 1 else nc.gpsimd
                    eng.tensor_tensor(
                        sm[:, t0:t0 + HU, :], te[:, :, :K],
                        recip[:].rearrange("p (t a) -> p t a", a=1)
                        .to_broadcast([128, HU, K]), op=OP.mult)

                def mm_unit(u):
                    b, h = divmod(u, 2)
                    t0 = u * HU
                    if h == 0:
                        pv2 = ps2.tile([128, 4 * K], F32, name="pv2",
                                       tag="pv")
                        pac = ps2.tile([K, 1], F32, name="pac", tag="pac")
                        state[b] = [pv2, pac, None]
                    pv2, pac, _ = state[b]
                    pv3 = pv2[:].rearrange("p (c k) -> p c k", c=4)
                    for c in range(4):
                        for i in range(HU):
                            t = t0 + i
                            nc.tensor.matmul(
                                pv3[:, c, :],
                                xh[:, t, c * 128:(c + 1) * 128],
                                sm[:, t, :],
                                start=(h == 0 and i == 0 and c == 0),
                                stop=False, skip_group_check=True)
                    # a_sum as a PE column: accumulating 1-row matmuls
                    for i in range(HU):
                        nc.tensor.matmul(pac[:], sm[:, t0 + i, :], ones16[:],
                                         start=(h == 0 and i == 0),
                                         stop=(h == 1 and i == HU - 1),
                                         skip_group_check=True)
                    if h == 1:
                        # -a_sum (fp16, SBUF; Act negating copy from PSUM)
                        # and diag(-a_sum) on gpsimd
                        acol = work.tile([K, 1], F16, name="acol", tag="acol")
                        nc.vector.tensor_scalar_mul(acol[:], pac[:], -1.0)
                        dga = vpost.tile([K, K], F16, name="dga", tag="dga")
                        nc.gpsimd.tensor_tensor(
                            dga[:], eye64[:], acol[:].to_broadcast([K, K]),
                            op=OP.mult)
                        state[b][2] = dga

                def close_batch(b):
                    pv2, _, dga = state[b]
                    pv3 = pv2[:].rearrange("p (c k) -> p c k", c=4)
                    for c in range(4):
                        nc.tensor.matmul(pv3[:, c, :], c2T[:, c, :], dga[:],
                                         start=False, stop=(c == 3),
                                         skip_group_check=True)

                def post_stage(b, last=False):
                    pv2, _, _ = state.pop(b)
                    pv3 = pv2[:].rearrange("p (c k) -> p c k", c=4)
                    v = vsb[:, b, :, :]
                    with nc.allow_low_precision("fp16 vlad drain"):
                        nc.scalar.activation(v, pv3[:], ACTF.Copy)
                    sq = vpost.tile([128, 4, K], F16, name="sq", tag="sq")
                    with nc.allow_low_precision("fp16 norm squares"):
                        if last:
                            # off the serial tail: square straight from
                            # PSUM on DVE, parallel with the Act drain
                            nc.vector.tensor_copy(sq[:], pv3[:])
                            nc.vector.tensor_tensor(sq[:], sq[:], sq[:],
                                                    op=OP.mult)
                        else:
                            nc.scalar.square(sq[:], v)
                    # intra-norm^2 per k: 4 accumulating ones-matmuls
                    pnrm = ps2.tile([1, K], F32, name="pnrm", tag="pnrm")
                    for c in range(4):
                        nc.tensor.matmul(pnrm[:], ones16[:], sq[:, c, :],
                                         start=(c == 0), stop=(c == 3),
                                         skip_group_check=True)
                    # 64*nrm2 (+tiny guard) for rn = 1/(8*sqrt(nrm2))
                    nc.vector.tensor_scalar(nrm2[:, b * K:(b + 1) * K],
                                            pnrm[:], 64.0, 64.0 * L2_EPS**2,
                                            op0=OP.mult, op1=OP.add)

                sn = persist.tile([1, b_loc * K], F32, name="sn")
                rn = persist.tile([1, b_loc * K], F16, name="rn")

                def finale_part(b0f, b1f):
                    # rn = 1/sqrt(64*nrm2); global L2 norm of the
                    # intra-normalized vlad is exactly sqrt(K)=8 (each of
                    # the K columns has unit norm; the eps guard can only
                    # fire on an exactly-zero column, which cannot occur
                    # for nondegenerate inputs). Batches 0..b_loc-2 are
                    # finished as soon as their norms land (the sqrt-set
                    # table is loaded right after the last exp); only the
                    # last batch's short chain sits at the end.
                    sl = slice(b0f * K, b1f * K)
                    nc.scalar.sqrt(sn[:, sl], nrm2[:, sl])
                    with nc.allow_low_precision("fp16 norm recip"):
                        nc.vector.reciprocal(rn[:, sl], sn[:, sl])
                    for b in range(b0f, b1f):
                        rn_sb = work.tile([128, K], F16, name="rn_sb",
                                          tag="rnsb")
                        nc.gpsimd.partition_broadcast(
                            rn_sb[:], rn[:, b * K:(b + 1) * K])
                        vb = vsb[:, b, :, :]
                        rb = (rn_sb[:].rearrange("p (a k) -> p a k", a=1)
                              .to_broadcast([128, 4, K]))
                        with nc.allow_low_precision("fp16 final scale"):
                            nc.vector.tensor_tensor(vb, vb, rb, op=OP.mult)

                def finale():
                    finale_part(b_loc - 1, b_loc)
                    # one output DMA for all batches: single HWDGE
                    # overhead, full-bandwidth descriptors
                    nc.sync.dma_start(
                        y.ap().rearrange("b p f -> p b f"), vsb[:])

                dum = work.tile([1, 1], F32, name="dum", tag="dum", bufs=1)
                NU = 2 * b_loc
                sm_aff(0)
                for u in range(NU):
                    if u + 1 < NU:
                        sm_aff(u + 1)
                        if u + 1 == NU - 1:
                            # dummy sqrt right behind the last exp: the
                            # sqrt-set activation-table load runs as soon
                            # as that exp retires, off the final serial
                            # chain (copy/square live in that set too)
                            nc.scalar.sqrt(dum[:], ones_row[:, :1])
                    sm_den(u)
                    mm_unit(u)
                    b, h = divmod(u, 2)
                    if h == 1 and b >= 1:
                        close_batch(b - 1)
                        post_stage(b - 1)
                        if b == b_loc - 1:
                            finale_part(0, b_loc - 1)
                close_batch(b_loc - 1)
                post_stage(b_loc - 1, last=True)
                finale()
    nc.compile()
    return nc


_CACHE = {}


def _get(b_loc, n_cores, with_collective):
    key = (b_loc, n_cores, with_collective)
    if key not in _CACHE:
        _CACHE[key] = build(b_loc, n_cores, with_collective)
    return _CACHE[key]


def make_in_maps(x, clusters, clusters2, bn_gamma, bn_beta, n_cores=N_CORES):
    B = x.shape[0]
    b_loc = B // n_cores
    shared = {
        "clusters": np.ascontiguousarray(clusters, np.float32),
        "clusters2": np.ascontiguousarray(
            np.asarray(clusters2).reshape(D, K), np.float32),
        "bn_gamma": np.ascontiguousarray(
            np.asarray(bn_gamma).reshape(1, KG), np.float32),
        "bn_beta": np.ascontiguousarray(
            np.asarray(bn_beta).reshape(1, KG), np.float32),
    }
    in_maps = []
    for i in range(n_cores):
        m = dict(shared)
        m["x"] = np.ascontiguousarray(
            np.asarray(x[i * b_loc:(i + 1) * b_loc]).reshape(
                b_loc * N_SEQ, D), np.float32)
        in_maps.append(m)
    return in_maps


def kernel(x, clusters, clusters2, bn_gamma, bn_beta):
    B, N, Dd = x.shape
    assert (N, Dd) == (N_SEQ, D) and B % N_CORES == 0
    b_loc = B // N_CORES
    nc = _get(b_loc, N_CORES, True)
    in_maps = make_in_maps(x, clusters, clusters2, bn_gamma, bn_beta)
    res = run_bass_kernel_spmd(nc, in_maps, core_ids=list(range(N_CORES)))
    out = np.concatenate([res.results[i]["y"] for i in range(N_CORES)], axis=0)
    # y is [b, p, c, k] fp16; vlad flat index is (128c+p)*K + k
    out = out.astype(np.float32)
    out = out.reshape(B, 128, 4, K).transpose(0, 2, 1, 3).reshape(B, D * K)
    return np.ascontiguousarray(out)
